# revision 17
# baseline (speedup 1.0000x reference)
"""Trainium2 Bass kernel for 16-head causal MHA (B=4, S=2048, D=1024).

Sharding: 8 cores = 4 batches x 2 head-groups (8 heads each).
Each core: QKV projection for its head-group, attention for 8 (b,h) pairs,
partial output projection.  Host sums the two partial out-projections per
batch and concatenates attn head slices.
"""

import sys

sys.path.insert(0, "/opt/trn_rl_repo")

from contextlib import ExitStack

import ml_dtypes
import numpy as np

import concourse.bacc as bacc
import concourse.mybir as mybir
import concourse.tile as tile
from concourse.bass_utils import run_bass_kernel_spmd

B, S, D, H, DK = 4, 2048, 1024, 16, 64
HPC = 8  # heads per core
GD = HPC * DK  # 512, head-group dim per core
NEG = -1e20
P = 128
NQB = S // P  # 16 q blocks
NCH = S // 512  # 4 chunks of 512

F32 = mybir.dt.float32
F32R = mybir.dt.float32r
BF16 = mybir.dt.bfloat16
Act = mybir.ActivationFunctionType

_cache = {}


def _build(causal: bool, reps: int = 1):
    nc = bacc.Bacc("TRN2", target_bir_lowering=False, debug=False, num_devices=8)

    def din(name, shape, dt=F32R):
        return nc.dram_tensor(name, shape, dt, kind="ExternalInput").ap()

    qT = din("qT", [D, S])
    kT = din("kT", [D, S])
    vT = din("vT", [D, S])
    wqT = din("wqT", [D, GD])
    wkT = din("wkT", [D, GD])
    wvT = din("wvT", [D, GD])
    woT = din("woT", [GD, D])
    bqc = din("bqc", [P, 4], F32)
    bkc = din("bkc", [P, 4], F32)
    bv = din("bv", [1, GD])
    bo = din("bo", [1, D])
    ones_h = din("ones_h", [1, P])
    ident_bf = din("ident_bf", [P, P], BF16)
    triu_neg = din("triu_neg", [P, P], BF16)  # strictly-upper NEG, [q,k] layout
    tril_neg = din("tril_neg", [P, P], BF16)  # strictly-lower NEG, [k,q] layout
    ident_f = din("ident_f", [P, P], F32)
    if not causal:
        smask = din("smask", [S, S], F32)  # additive bias, [q,k]
        smaskT = din("smaskT", [S, S], F32)  # additive bias, [k,q]

    attn = nc.dram_tensor("attn", [HPC, S, S], F32, kind="ExternalOutput").ap()
    out = nc.dram_tensor("out", [4, S, D], F32, kind="ExternalOutput").ap()
    rtd = nc.dram_tensor("rtd", [4, 2, NQB, P], F32, kind="Internal").ap()

    with tile.TileContext(nc) as tc, ExitStack() as ctx:
        if reps > 1:
            ctx.enter_context(tc.For_i(0, reps, 1))
        consts = ctx.enter_context(tc.tile_pool(name="consts", bufs=1))

        def cload(ap_in, shape, dt, cname):
            t = consts.tile(shape, dt, tag=cname, name=cname)
            nc.sync.dma_start(t[:], ap_in[:])
            return t

        ident_bf_s = cload(ident_bf, [P, P], BF16, "c_identbf")
        triu_s = cload(triu_neg, [P, P], BF16, "c_triu")
        tril_s = cload(tril_neg, [P, P], BF16, "c_tril")
        ident_f_s = cload(ident_f, [P, P], F32, "c_identf")
        bq_s = cload(bqc, [P, 4], F32, "c_bq")
        bk_s = cload(bkc, [P, 4], F32, "c_bk")
        bv_s = cload(bv, [1, GD], F32R, "c_bv")
        bo_s = cload(bo, [1, D], F32R, "c_bo")
        ones_s = cload(ones_h, [1, P], F32R, "c_ones")

        # persistent activations
        qk_pool = ctx.enter_context(tc.tile_pool(name="qk", bufs=8))
        v_pool = ctx.enter_context(tc.tile_pool(name="v", bufs=16))
        wo_pool = ctx.enter_context(tc.tile_pool(name="wo", bufs=4))
        WO = []
        for pb_ in range(4):
            w = wo_pool.tile([P, D], F32R, tag="wo", name=f"wo{pb_}")
            nc.scalar.dma_start(w[:], woT[pb_ * P : (pb_ + 1) * P, :])
            WO.append(w)
        QT = {}  # (pair, half) -> [128, 1024]; pair p rows: h0=2p (0:64), h1 (64:128)
        KT = {}
        V = []  # 16 blocks [128, GD], token-major

        # ---------- Phase 1: projections ----------
        with tc.tile_pool(name="xt", bufs=9) as xt_pool, tc.tile_pool(
            name="w1", bufs=8
        ) as w_pool, tc.tile_pool(name="ps_qk", bufs=2, space="PSUM") as ps_qk, tc.tile_pool(
            name="ps_v", bufs=2, space="PSUM"
        ) as ps_v:

            engs = [nc.sync, nc.scalar, nc.gpsimd]

            def load_blocks(src, nblk, width, pool, tag):
                ts = []
                k = 0
                for i in range(nblk):
                    t = pool.tile([P, width], F32R, tag=tag)
                    hw_ = width // 2
                    for h2 in range(2):
                        engs[k % 3].dma_start(
                            t[:, h2 * hw_ : (h2 + 1) * hw_],
                            src[i * P : (i + 1) * P, h2 * hw_ : (h2 + 1) * hw_],
                        )
                        k += 1
                    ts.append(t)
                return ts

            # Q then K: out-transposed [outdim, tok], stored per 1024-token half
            for (xsrc, wsrc, bias_s, dst, tg) in (
                (qT, wqT, bq_s, QT, "qt"),
                (kT, wkT, bk_s, KT, "kt"),
            ):
                xts = load_blocks(xsrc, 8, S, xt_pool, "xt")
                wts = load_blocks(wsrc, 8, GD, w_pool, "w1")
                for ob in range(4):
                    for half in range(2):
                        ps = ps_qk.tile([P, 1024], F32, tag="qk")
                        for ch in range(2):
                            for ib in range(8):
                                nc.tensor.matmul(
                                    ps[:, ch * 512 : (ch + 1) * 512],
                                    wts[ib][:, ob * P : (ob + 1) * P],
                                    xts[ib][
                                        :,
                                        half * 1024
                                        + ch * 512 : half * 1024
                                        + (ch + 1) * 512,
                                    ],
                                    start=(ib == 0),
                                    stop=(ib == 7),
                                    skip_group_check=True,
                                )
                        o = qk_pool.tile([P, 1024], F32R, tag=tg)
                        nc.vector.tensor_scalar_add(o[:], ps[:], bias_s[:, ob : ob + 1])
                        dst[(ob, half)] = o

            # V: token-major [tok, outdim]
            vts = load_blocks(vT, 8, S, xt_pool, "xt")
            wvs = load_blocks(wvT, 8, GD, w_pool, "w1")
            for tb in range(16):
                ps = ps_v.tile([P, GD], F32, tag="v")
                for ib in range(8):
                    nc.tensor.matmul(
                        ps[:],
                        vts[ib][:, tb * P : (tb + 1) * P],
                        wvs[ib][:],
                        start=(ib == 0),
                        stop=False,
                        skip_group_check=True,
                    )
                nc.tensor.matmul(
                    ps[:], ones_s[:], bv_s[:], start=False, stop=True,
                    skip_group_check=True,
                )
                o = v_pool.tile([P, GD], F32R, tag="v")
                nc.vector.tensor_copy(o[:], ps[:])
                V.append(o)

        def qslice(d, pair, hh, lo, hi):
            # columns [lo:hi) of the virtual [64, S] row block for head hh of pair
            half = lo // 1024
            assert (hi - 1) // 1024 == half
            return d[(pair, half)][
                hh * 64 : (hh + 1) * 64, lo - half * 1024 : hi - half * 1024
            ]

        # ---------- Phase 2: attention per head-pair ----------
        mk_ctx = ExitStack()
        with tc.tile_pool(name="pt", bufs=6) as pt_pool, tc.tile_pool(
            name="pbuf", bufs=3
        ) as p_pool, tc.tile_pool(name="otu", bufs=2) as otu_pool, tc.tile_pool(
            name="small", bufs=8
        ) as small_pool, tc.tile_pool(name="rrep", bufs=2) as rrep_pool, tc.tile_pool(
            name="xc", bufs=3
        ) as xc_pool, tc.tile_pool(name="zp", bufs=6) as zp_pool, tc.tile_pool(
            name="ps_sc", bufs=2, space="PSUM"
        ) as ps_sc, tc.tile_pool(name="ps_ov", bufs=2, space="PSUM") as ps_ov:
            mk_pool = (
                mk_ctx.enter_context(tc.tile_pool(name="mk", bufs=8))
                if not causal
                else None
            )
            LAG = 2  # attnV trails sT/exp by LAG kblocks so PE never waits on ACT
            pending = []

            def flush_pending():
                while pending:
                    pair_tail(*pending.pop(0))

            for pair in range(4):
                rz = [small_pool.tile([P, NQB], F32, tag="rz", name=f"rz{pair}_{_h}") for _h in range(2)]
                otu = [
                    otu_pool.tile([64, S], F32, tag="otu", name=f"otu{pair}_{_h}")
                    for _h in range(2)
                ]

                for c in range(NCH):
                    jmax = (4 * c + 3) if causal else (NQB - 1)
                    ps_o = [
                        ps_ov.tile([64, 512], F32, tag="ov", name=f"pso{pair}_{c}_{_h}")
                        for _h in range(2)
                    ]
                    ptts = {}  # j -> (off, [pt_h0, pt_h1])

                    def attn_v(j):
                        off, ptt = ptts.pop(j)
                        for hh in range(2):
                            h = 2 * pair + hh
                            nc.tensor.matmul(
                                ps_o[hh][:, off:512],
                                V[j][:, h * DK : (h + 1) * DK],
                                ptt[hh][:, off:512],
                                start=(j == 0),
                                stop=(j == jmax),
                                skip_group_check=True,
                            )

                    for j in range(jmax + 1):
                        off = max(0, j * P - c * 512) if causal else 0
                        has_diag = causal and j >= 4 * c
                        pst = []
                        for hh in range(2):
                            ps_t = ps_sc.tile([P, 512], F32, tag="st")
                            nc.tensor.matmul(
                                ps_t[:, off:512],
                                qslice(KT, pair, hh, j * P, (j + 1) * P),
                                qslice(QT, pair, hh, c * 512 + off, (c + 1) * 512),
                                start=True,
                                stop=not has_diag,
                                skip_group_check=True,
                            )
                            if has_diag:
                                nc.tensor.matmul(
                                    ps_t[:, off : off + P],
                                    ident_bf_s[:],
                                    tril_s[:],
                                    start=False,
                                    stop=True,
                                    skip_group_check=True,
                                )
                            pst.append(ps_t)
                        if not causal:
                            for hh in range(2):
                                m = mk_pool.tile([P, 512], F32, tag="mk")
                                nc.sync.dma_start(
                                    m[:],
                                    smaskT[j * P : (j + 1) * P, c * 512 : (c + 1) * 512],
                                )
                                nc.vector.tensor_add(pst[hh][:], pst[hh][:], m[:])
                        ptt = []
                        for hh in range(2):
                            pt = pt_pool.tile([P, 512], F32R, tag="pt")
                            nc.scalar.activation(
                                pt[:, off:512], pst[hh][:, off:512], Act.Exp
                            )
                            ptt.append(pt)
                        ptts[j] = (off, ptt)
                        if j >= LAG:
                            attn_v(j - LAG)
                    for j in range(max(0, jmax + 1 - LAG), jmax + 1):
                        attn_v(j)
                    for hh in range(2):
                        nc.vector.tensor_copy(
                            otu[hh][:, c * 512 : (c + 1) * 512], ps_o[hh][:]
                        )

                    if c == 1:
                        flush_pending()
                    # --- S path for qblocks of this chunk ---
                    # 1024-wide psum chunks: one exp+accum per chunk
                    for r in range(4):
                        i = 4 * c + r
                        lk = (i + 1) * P if causal else S
                        ncc2 = (lk + 1023) // 1024
                        for hh in range(2):
                            pb = p_pool.tile([P, S], F32, tag="p")
                            zparts = zp_pool.tile([P, 2], F32, tag="zp")
                            for cc2 in range(ncc2):
                                wS2 = min(1024, lk - cc2 * 1024)
                                ps_s = ps_sc.tile([P, 1024], F32, tag="sc")
                                for h5 in range(2):
                                    lo = cc2 * 1024 + h5 * 512
                                    wS = min(512, lk - lo)
                                    if wS <= 0:
                                        continue
                                    is_diag = causal and (i * P) < lo + wS
                                    nc.tensor.matmul(
                                        ps_s[:, h5 * 512 : h5 * 512 + wS],
                                        qslice(QT, pair, hh, i * P, (i + 1) * P),
                                        qslice(KT, pair, hh, lo, lo + wS),
                                        start=True,
                                        stop=not is_diag,
                                        skip_group_check=True,
                                    )
                                    if is_diag:
                                        dcol = i * P - cc2 * 1024
                                        nc.tensor.matmul(
                                            ps_s[:, dcol : dcol + P],
                                            ident_bf_s[:],
                                            triu_s[:],
                                            start=False,
                                            stop=True,
                                            skip_group_check=True,
                                        )
                                    if not causal:
                                        m = mk_pool.tile([P, 512], F32, tag="mk")
                                        nc.sync.dma_start(
                                            m[:],
                                            smask[i * P : (i + 1) * P, lo : lo + wS],
                                        )
                                        nc.vector.tensor_add(
                                            ps_s[:, h5 * 512 : h5 * 512 + wS],
                                            ps_s[:, h5 * 512 : h5 * 512 + wS],
                                            m[:, 0:wS],
                                        )
                                nc.scalar.activation(
                                    pb[:, cc2 * 1024 : cc2 * 1024 + wS2],
                                    ps_s[:, 0:wS2],
                                    Act.Exp,
                                    accum_out=zparts[:, cc2 : cc2 + 1],
                                )
                            zs = zp_pool.tile([P, 1], F32, tag="zs")
                            nc.vector.reduce_sum(
                                zs[:, 0:1], zparts[:, 0:ncc2], mybir.AxisListType.X
                            )
                            nc.vector.reciprocal(rz[hh][:, i : i + 1], zs[:, 0:1])
                            nc.vector.tensor_scalar_mul(
                                pb[:, 0:lk], pb[:, 0:lk], rz[hh][:, i : i + 1]
                            )
                            h = 2 * pair + hh
                            eng = nc.sync if (i + hh) % 2 == 0 else nc.gpsimd
                            eng.dma_start(
                                attn[h, i * P : (i + 1) * P, 0:lk], pb[:, 0:lk]
                            )

                # --- normalize outT, assemble X, per-pair out projection ---
                # (deferred: emitted during the next pair's compute so the PE
                # doesn't stall on this pair's S-path chain)
                def pair_tail(pair, rz, otu):
                    for hh in range(2):
                        nc.sync.dma_start(
                            rtd[pair, hh].transpose([1, 0]), rz[hh][:]
                        )
                    xp = xc_pool.tile([P, S], F32R, tag="xp", bufs=2, name=f"xp{pair}")
                    for c in range(NCH):
                        for hh in range(2):
                            rr = rrep_pool.tile(
                                [64, 512], F32, tag="rr", name=f"rr{pair}_{c}_{hh}"
                            )
                            nc.gpsimd.dma_start(
                                rr[:].rearrange("p (r q) -> p r q", r=4),
                                rtd[pair, hh, 4 * c : 4 * c + 4, :].partition_broadcast(
                                    64
                                ),
                            )
                            if hh == 0:
                                nc.vector.tensor_mul(
                                    xp[0:64, c * 512 : (c + 1) * 512],
                                    otu[0][:, c * 512 : (c + 1) * 512],
                                    rr[:],
                                )
                            else:
                                x1 = xc_pool.tile(
                                    [64, 512], F32R, tag="x1", bufs=2,
                                    name=f"x1{pair}_{c}",
                                )
                                nc.vector.tensor_mul(
                                    x1[:], otu[1][:, c * 512 : (c + 1) * 512], rr[:]
                                )
                                nc.gpsimd.dma_start(
                                    xp[64:P, c * 512 : (c + 1) * 512], x1[:]
                                )
                    # out-proj partial for this pair
                    for tb in range(16):
                        osb = xc_pool.tile(
                            [P, D], F32, tag="o3", bufs=2, name=f"osb{pair}_{tb}"
                        )
                        ps = ps_sc.tile(
                            [P, 1024], F32, tag="sc", name=f"pp{pair}_{tb}"
                        )
                        for nch in range(2):
                            nc.tensor.matmul(
                                ps[:, nch * 512 : (nch + 1) * 512],
                                xp[:, tb * P : (tb + 1) * P],
                                WO[pair][:, nch * 512 : (nch + 1) * 512],
                                start=True,
                                stop=pair != 0,
                                skip_group_check=True,
                            )
                            if pair == 0:
                                nc.tensor.matmul(
                                    ps[:, nch * 512 : (nch + 1) * 512],
                                    ones_s[:],
                                    bo_s[:, nch * 512 : (nch + 1) * 512],
                                    start=False,
                                    stop=True,
                                    skip_group_check=True,
                                )
                        nc.vector.tensor_copy(osb[:], ps[:])
                        eng = nc.sync if tb % 2 == 0 else nc.gpsimd
                        eng.dma_start(out[pair, tb * P : (tb + 1) * P, :], osb[:])

                pending.append((pair, rz, otu))
            flush_pending()
        mk_ctx.close()

    nc.compile()
    return nc


def _get_nc(causal, reps=1):
    key = (causal, reps)
    if key not in _cache:
        _cache[key] = _build(causal, reps)
    return _cache[key]


def kernel(q, k, v, mask, Wq, bq, Wk, bk, Wv, bv, Wo, bo, _trace=False):
    q = np.asarray(q, np.float32)
    k = np.asarray(k, np.float32)
    v = np.asarray(v, np.float32)
    mask2d = np.asarray(mask).reshape(S, S)
    causal = bool(np.array_equal(mask2d != 0, np.tril(np.ones((S, S), bool))))
    nc = _get_nc(causal)

    ident = np.eye(P, dtype=np.float32)
    ident_bf = ident.astype(ml_dtypes.bfloat16)
    triu_neg = np.triu(np.full((P, P), NEG, np.float32), 1).astype(ml_dtypes.bfloat16)
    tril_neg = np.tril(np.full((P, P), NEG, np.float32), -1).astype(ml_dtypes.bfloat16)
    ones_h = np.ones((1, P), np.float32)
    if not causal:
        sm = np.where(mask2d == 0, np.float32(NEG), np.float32(0.0))
        smT = np.ascontiguousarray(sm.T)

    in_maps = []
    for c in range(8):
        b, g = c // 2, c % 2
        sl = slice(g * GD, (g + 1) * GD)
        m = {
            "qT": np.ascontiguousarray(q[b].T),
            "kT": np.ascontiguousarray(k[b].T),
            "vT": np.ascontiguousarray(v[b].T),
            "wqT": np.ascontiguousarray((np.asarray(Wq)[sl, :] / 8.0).T).astype(np.float32),
            "wkT": np.ascontiguousarray(np.asarray(Wk)[sl, :].T).astype(np.float32),
            "wvT": np.ascontiguousarray(np.asarray(Wv)[sl, :].T).astype(np.float32),
            "woT": np.ascontiguousarray(np.asarray(Wo)[:, sl].T).astype(np.float32),
            "bqc": np.ascontiguousarray(
                (np.asarray(bq)[sl] / 8.0).reshape(4, P).T
            ).astype(np.float32),
            "bkc": np.ascontiguousarray(np.asarray(bk)[sl].reshape(4, P).T).astype(
                np.float32
            ),
            "bv": np.asarray(bv)[sl].reshape(1, GD).astype(np.float32),
            "bo": (np.asarray(bo) * (1.0 if g == 0 else 0.0))
            .reshape(1, D)
            .astype(np.float32),
            "ones_h": ones_h,
            "ident_bf": ident_bf,
            "triu_neg": triu_neg,
            "tril_neg": tril_neg,
            "ident_f": ident,
        }
        if not causal:
            m["smask"] = sm
            m["smaskT"] = smT
        in_maps.append(m)

    res = run_bass_kernel_spmd(nc, in_maps, core_ids=list(range(8)), trace=_trace)

    out_full = np.zeros((B, S, D), np.float32)
    attn_full = np.zeros((B, H, S, S), np.float32)
    for c in range(8):
        b, g = c // 2, c % 2
        out_full[b] += res.results[c]["out"].sum(axis=0)
        attn_full[b, g * HPC : (g + 1) * HPC] = res.results[c]["attn"]
    if _trace:
        kernel._last = res
    return out_full, attn_full
